# revision 1
# baseline (speedup 1.0000x reference)
"""Mistral GQA self-attention block on 8 Trainium2 NeuronCores (Bass/Tile).

Sharding: tensor-parallel over heads. Core m owns q-heads 4m..4m+3 and
kv-head m (GQA group-aligned), Wq/Wk/Wv column-sharded, Wo row-sharded.
Each core computes a full-size [B*T, H*D] partial of the output
projection; the host sums the 8 partials (the Wo row-parallel reduce).

Per-core kernel layout (feature-major, [feature, token] for q/k):
  phase 1: QKV projection. q heads + k head come out feature-major
           (qkvT[f, t] = W_shard @ x.T) with RoPE applied in place:
           the half-rotation is two partition-swap copies plus a
           sign-folded sin table -- no PE rotation matmul. v is
           produced directly in [t, d] token-major layout (x sub-tile
           as the stationary operand) so attention needs no
           transposes. DMAs are batched in multi-k-tile chunks sized
           so tb0 streaming stays ahead of the PE, under the
           per-transfer HWDGE/SP issue overhead.
  phase 2+3 (merged, shared 8-bank psum tag pool, no pool barriers):
           attention per (batch, head). Scores are computed TRANSPOSED
           (st[tk, tq] = k_tile.T @ q) so the exp'd probabilities come
           out of the scalar engine already in the [tk, tq] layout the
           PV matmul needs. The causal diagonal blocks are masked by a
           0/1-triangle multiply on the (otherwise idle) Pool engine
           instead of a PE -1e9 preload. The softmax denominator is
           accumulated with a ones-vector matmul and applied to the
           (8x smaller) output tile instead of to p. The output
           projection (partial[t, e] = oT.T @ WoT_shard, fp16 out,
           host-summed in fp32) is emitted as [128,512]-psum groups,
           interleaved into the b=1 attention stream as PE filler
           where the scalar engine (exp) is the local bottleneck; Wo
           tiles are prefetched during attention.

Matmul dtype via BASS_MM_DTYPE = f32 | f32r | bf16 | f16 (default:
fp16 stores, fp32 psum accumulation -- same PE rate as bf16, 8x finer
mantissa).
"""

import os
import sys

import numpy as np

for _p in ("/opt/trn_rl_repo", "/root/.axon_site/_ro/trn_rl_repo"):
    if os.path.isdir(_p):
        if _p not in sys.path:
            sys.path.insert(0, _p)
        break

import ml_dtypes  # noqa: E402

B, T, H, D = 2, 1024, 32, 128
KV = 8
M = 8                 # cores
QH = H // M           # q heads per core
FT = QH + 1           # feature-major tiles per core: 4 q, 1 k (v separate)
CD = H * D            # contraction dim 4096
CT = CD // 128        # 32 c-tiles
BT = B * T            # 2048 tokens
NTQ = T // D          # 8 tq/tk tiles per batch
NEG = -1e9
SCALE = 1.0 / np.sqrt(D)

MM_MODE = os.environ.get("BASS_MM_DTYPE", "f16")
assert MM_MODE in ("f32", "f32r", "bf16", "f16")

# k-tile chunk sizes for the batched weight/x DMAs (sum = CT).
# Sized so chunk c of w+x lands before the tb0 k-loop consumes it.
CHUNKS = (1, 1, 2, 2, 3, 3, 4, 4, 4, 4, 4)
_CH_OFF = [sum(CHUNKS[:i]) for i in range(len(CHUNKS))]
N_WARM = int(os.environ.get("BASS_N_WARM", "28"))


# ---------------------------------------------------------------- host prep

def _tf32_round(a):
    """Round fp32 to the TF32 (1+8+10) representable set, RNE."""
    u = np.ascontiguousarray(a, np.float32).view(np.uint32)
    u = (u + 0x0FFF + ((u >> 13) & 1)) & np.uint32(0xFFFFE000)
    return u.view(np.float32)


def _store(a):
    """Cast a host array to the on-device storage dtype for matmul inputs."""
    if MM_MODE == "f16":
        return np.ascontiguousarray(np.asarray(a, np.float32)).astype(
            np.float16)
    if MM_MODE == "bf16":
        return np.ascontiguousarray(np.asarray(a, np.float32)).astype(
            ml_dtypes.bfloat16)
    if MM_MODE == "f32r":
        return _tf32_round(np.asarray(a, np.float32))
    return np.ascontiguousarray(np.asarray(a), np.float32)


def host_prep(stm, Wq, Wk, Wv, Wo, cos, sin, mask_b):
    """Build the 8 per-core input maps."""
    x = np.ascontiguousarray(np.asarray(stm, np.float32).reshape(BT, CD))
    xT = _store(x.T)                                     # [4096, 2048]
    cosT = np.ascontiguousarray(cos[0, :, 0, :].T, np.float32)   # [128, 1024]
    sinS = np.ascontiguousarray(sin[0, :, 0, :].T, np.float32).copy()
    sinS[:D // 2] = -sinS[:D // 2]   # sign of the half-rotation folded in
    triu = _store(np.triu(np.ones((D, D), np.float32)))
    ones1 = _store(np.ones((D, D), np.float32))

    in_maps = []
    for m in range(M):
        wq = Wq[m * QH * D:(m + 1) * QH * D]             # [512, 4096]
        wk = Wk[m * D:(m + 1) * D]                       # [128, 4096]
        wv = Wv[m * D:(m + 1) * D]                       # [128, 4096]
        wqkvT = _store(np.concatenate([wq, wk, wv], 0).T)  # [4096, 768]
        woT = _store(Wo[:, m * QH * D:(m + 1) * QH * D].T)  # [512, 4096]
        in_maps.append({
            "xT": xT, "wqkvT": wqkvT, "woT": woT,
            "cosT": cosT, "sinS": sinS, "triu": triu,
            "ones1": ones1,
        })
    return in_maps


# ---------------------------------------------------------------- bass prog

def _build_nc(causal=True, phases=(1, 2, 3)):
    import concourse.tile as tile
    from concourse import bacc, mybir

    dt_store = {"f16": mybir.dt.float16,
                "bf16": mybir.dt.bfloat16,
                "f32r": mybir.dt.float32r,
                "f32": mybir.dt.float32}[MM_MODE]
    f32 = mybir.dt.float32
    f16 = mybir.dt.float16

    nc = bacc.Bacc("TRN2", target_bir_lowering=False, debug=False)

    xT_d = nc.dram_tensor("xT", [CD, BT], dt_store, kind="ExternalInput")
    wqkvT_d = nc.dram_tensor("wqkvT", [CD, (FT + 1) * D], dt_store,
                             kind="ExternalInput")
    woT_d = nc.dram_tensor("woT", [QH * D, CD], dt_store, kind="ExternalInput")
    cosT_d = nc.dram_tensor("cosT", [D, T], f32, kind="ExternalInput")
    sinS_d = nc.dram_tensor("sinS", [D, T], f32, kind="ExternalInput")
    triu_d = nc.dram_tensor("triu", [D, D], dt_store, kind="ExternalInput")
    ones1_d = nc.dram_tensor("ones1", [D, D], dt_store, kind="ExternalInput")
    outp_d = nc.dram_tensor("outp", [BT, CD], f16, kind="ExternalOutput")
    dbg = bool(os.environ.get("BASS_DEBUG_QKV"))
    if dbg:
        qkv_dbg_d = nc.dram_tensor("qkv_dbg", [FT * 128, B * T], f32,
                                   kind="ExternalOutput")
        v_dbg_d = nc.dram_tensor("v_dbg", [128, B * T], f32,
                                 kind="ExternalOutput")

    add = mybir.AluOpType.add
    mult = mybir.AluOpType.mult
    Exp = mybir.ActivationFunctionType.Exp
    half = D // 2

    def chunks_for_j(j):
        """Valid tq chunk ranges [(lo, hi)] for k-tile j (<=512 wide,
        psum-bank aligned ends)."""
        if not causal:
            return [(0, 512), (512, 1024)]
        w0 = D * j
        out = []
        if w0 < 512:
            out.append((w0, 512))
        out.append((max(512, w0), 1024))
        return out

    def phase1(tc, psum, qkvT_sb, v_sb, issue_consts):
        with tc.tile_pool(name="wqkv", bufs=1) as wpool, \
             tc.tile_pool(name="xin", bufs=2) as xpool, \
             tc.tile_pool(name="rope", bufs=1) as rpool:
            wqkvT_r = wqkvT_d.ap().rearrange("(k p) f -> p k f", p=128)
            xT_r = xT_d.ap().rearrange("(k p) t -> p k t", p=128)
            w_c = [wpool.tile([128, n, (FT + 1) * D], dt_store,
                              tag=f"w{ci}", name=f"w_{ci}")
                   for ci, n in enumerate(CHUNKS)]

            # PE p-state warmup: spin on a zeroed tile into a scratch psum
            # bank while the first DMAs land, so the tensor engine is at
            # full clock when real data arrives (idle resets the ramp)
            warm = rpool.tile([128, 128], dt_store, tag="warm")
            nc.vector.memset(warm[:], 0)
            wps = psum.tile([128, 512], f32, tag="b6", name="warmup_ps")
            for _ in range(N_WARM):
                nc.tensor.matmul(wps[:, 0:128], lhsT=warm[:], rhs=warm[:],
                                 start=True, stop=True)

            def issue_x(tb):
                xc = []
                for ci, n in enumerate(CHUNKS):
                    t = xpool.tile([128, n, 512], dt_store, tag=f"x{ci}",
                                   name=f"x_{ci}_{tb}")
                    nc.sync.dma_start(
                        t[:], xT_r[:, _CH_OFF[ci]:_CH_OFF[ci] + n,
                                   tb * 512:(tb + 1) * 512])
                    xc.append(t)
                return xc

            def wslice(k, ft):
                ci = 0
                while k >= _CH_OFF[ci] + CHUNKS[ci]:
                    ci += 1
                return w_c[ci][:, k - _CH_OFF[ci], ft * D:(ft + 1) * D]

            def xslice(xc, k, lo=0, hi=512):
                ci = 0
                while k >= _CH_OFF[ci] + CHUNKS[ci]:
                    ci += 1
                return xc[ci][:, k - _CH_OFF[ci], lo:hi]

            def mm_v(pv, xc):
                # one accumulation chain at a time: concurrent psum
                # accumulation groups within one bank are illegal
                for i in range(4):
                    for k in range(CT):
                        nc.tensor.matmul(
                            pv[:, i * D:(i + 1) * D],
                            lhsT=xslice(xc, k, i * D, (i + 1) * D),
                            rhs=wslice(k, FT),
                            start=(k == 0), stop=(k == CT - 1))

            def mm_ft(ps, xc, k, ft):
                nc.tensor.matmul(
                    ps[ft][:], lhsT=wslice(k, ft), rhs=xslice(xc, k),
                    start=(k == 0), stop=(k == CT - 1))

            def rope_qk(tb, ps):
                """Drain the q/k psum accumulators and apply RoPE. Emitted
                BEFORE the v chains so Act/DVE drain the banks while the PE
                runs the v matmuls (engines are in-order)."""
                t0 = (tb % 2) * 512
                csl = cosT_sb[:, t0:t0 + 512]
                ssl = sinS_sb[:, t0:t0 + 512]
                bb, tsl = tb // 2, slice(t0, t0 + 512)
                qraw = [rpool.tile([128, 512], f32, tag=f"qraw{ft}",
                                   name=f"qraw_{ft}_{tb}")
                        for ft in range(FT)]
                for ft in range(FT):
                    nc.scalar.copy(qraw[ft][:], ps[ft][:])
                for ft in range(FT):
                    dtile = qkvT_sb[ft][bb]
                    dst = dtile[:, tsl]
                    q = qraw[ft]
                    # half-rotation via partition-swap copies, then
                    # dst = swap(q) * sinS + q * cos -- all on the vector
                    # engine so the chain has no cross-engine ping-pong
                    nc.vector.tensor_copy(dtile[0:half, tsl], q[half:128, :])
                    nc.vector.tensor_copy(dtile[half:128, tsl], q[0:half, :])
                    tmp = rpool.tile([128, 512], f32, tag="rtmp")
                    nc.vector.tensor_tensor(dst, dst, ssl, mult)
                    nc.vector.tensor_tensor(tmp[:], q[:], csl, mult)
                    nc.vector.tensor_tensor(dst, dst, tmp[:], add)

            def drain_v(tb, pv):
                tsl = slice((tb % 2) * 512, (tb % 2) * 512 + 512)
                nc.scalar.copy(v_sb[tb // 2][:, tsl], pv[:])

            for tb in range(BT // 512):
                ps = [psum.tile([128, 512], f32, tag=f"b{ft}",
                                name=f"ps_qkv{ft}_{tb}")
                      for ft in range(FT)]
                pv = psum.tile([128, 512], f32, tag="b5", name=f"ps_v_{tb}")
                if tb == 0:
                    # interleave w / x chunk DMAs, then consts + wo
                    for ci, n in enumerate(CHUNKS):
                        nc.sync.dma_start(
                            w_c[ci][:], wqkvT_r[:, _CH_OFF[ci]:_CH_OFF[ci] + n])
                        xt = xpool.tile([128, n, 512], dt_store, tag=f"x{ci}",
                                        name=f"x_{ci}_0")
                        nc.sync.dma_start(
                            xt[:], xT_r[:, _CH_OFF[ci]:_CH_OFF[ci] + n, 0:512])
                        if ci == 0:
                            xc = []
                        xc.append(xt)
                    issue_consts()
                    # k-outer while the chunk DMAs stream in
                    for k in range(CT):
                        for ft in range(FT):
                            mm_ft(ps, xc, k, ft)
                else:
                    # ft-outer: each chain runs 6.8us, hiding the psum
                    # drain of the previous tb's rope
                    for ft in range(FT):
                        for k in range(CT):
                            mm_ft(ps, xc, k, ft)
                rope_qk(tb, ps)
                mm_v(pv, xc)
                if tb < BT // 512 - 1:
                    xc = issue_x(tb + 1)
                drain_v(tb, pv)

    def phase23(tc, psum, ppool, spool, qkvT_sb, v_sb, oT_sb, w2):
        with tc.tile_pool(name="oout", bufs=4) as opool:
            # --- output-projection groups: [128,512] psum chains over the
            # 4 head-tiles, emitted as PE filler inside phase 2 (b=1) and
            # in bulk afterwards. Rotate over the shared psum bank tags.
            NG = (BT // 128) * 8           # (tt) x (eh, g2) groups
            gstate = {"g": 0, "ot": None, "c": 0}
            tags4 = ["b6", "b7", "b4", "b5"]
            tags8 = [f"b{i}" for i in range(8)]

            def emit_groups(count, deep=False):
                tags = tags8 if deep else tags4
                for _ in range(count):
                    gi = gstate["g"]
                    if gi >= NG:
                        return
                    gstate["g"] = gi + 1
                    tt, rem = divmod(gi, 8)
                    eh, g2 = divmod(rem, 4)
                    if g2 == 0:
                        gstate["ot"] = opool.tile([128, 2048], f16, tag="ot",
                                                  name=f"ot_{tt}_{eh}")
                    ot = gstate["ot"]
                    tag = tags[gstate["c"] % len(tags)]
                    gstate["c"] += 1
                    pps = psum.tile([128, 512], f32, tag=tag,
                                    name=f"ps3_{tt}_{eh}_{g2}")
                    for ht in range(QH):
                        nc.tensor.matmul(
                            pps[:],
                            lhsT=oT_sb[ht][tt // NTQ][
                                :, (tt % NTQ) * D:(tt % NTQ + 1) * D],
                            rhs=w2[eh * QH + ht][:, g2 * 512:(g2 + 1) * 512],
                            start=(ht == 0), stop=(ht == QH - 1))
                    osl = slice(g2 * 512, (g2 + 1) * 512)
                    if gi % 2 == 0:
                        nc.vector.tensor_copy(ot[:, osl], pps[:])
                    else:
                        nc.scalar.copy(ot[:, osl], pps[:])
                    if gi >= NG - 4:
                        # tail groups: per-group DMA so the final transfer
                        # (which gates the drain) is 4x smaller
                        nc.sync.dma_start(
                            outp_d.ap()[tt * 128:(tt + 1) * 128,
                                        eh * 2048 + g2 * 512:
                                        eh * 2048 + (g2 + 1) * 512],
                            ot[:, osl])
                    elif g2 == 3:
                        nc.sync.dma_start(
                            outp_d.ap()[tt * 128:(tt + 1) * 128,
                                        eh * 2048:(eh + 1) * 2048],
                            ot[:])

            for b in range(B):
                vT = v_sb[b]
                for h in range(QH):
                    qsl = qkvT_sb[h][b][:]
                    ksl = qkvT_sb[QH][b][:]
                    rs_c = [psum.tile([128, 512], f32, tag=f"b{6 + c}",
                                      name=f"rs_{b}_{h}_{c}")
                            for c in range(2)]
                    pT = ppool.tile([128, NTQ, T], dt_store, tag="pT")
                    st_tags = ("b0", "b1", "b2", "b3")
                    nst = 0
                    recip = spool.tile([128, T], f32, tag="recip")
                    for j in range(NTQ):
                        for ci, (lo, hi) in enumerate(chunks_for_j(j)):
                            W = hi - lo
                            st = psum.tile([128, 512], f32,
                                           tag=st_tags[nst % 4],
                                           name=f"st_{b}_{h}_{j}_{ci}")
                            nst += 1
                            diag = causal and ci == 0
                            nc.tensor.matmul(
                                st[:, :W],
                                lhsT=ksl[:, j * D:(j + 1) * D],
                                rhs=qsl[:, lo:hi],
                                start=True, stop=True)
                            nc.scalar.activation(
                                pT[:, j, lo:hi], st[:, :W], Exp,
                                scale=float(SCALE))
                            if diag:
                                # zero the causally-invalid lower triangle
                                # of the diagonal block on the (idle) Pool
                                # engine instead of a PE -1e9 preload
                                nc.gpsimd.tensor_tensor(
                                    pT[:, j, lo:lo + D], pT[:, j, lo:lo + D],
                                    triu_sb[:], mult)
                            last_j = (min(NTQ, hi // D) - 1) if causal \
                                else NTQ - 1
                            c = 0 if hi <= 512 else 1
                            nc.tensor.matmul(
                                rs_c[c][:, lo - c * 512:hi - c * 512],
                                lhsT=ones_sb[:],
                                rhs=pT[:, j, lo:hi],
                                start=(j == 0),
                                stop=(j == last_j))
                            if causal and c == 0 and j == last_j:
                                # c0's denominator is final here; compute its
                                # reciprocal now so the po matmuls after the
                                # j loop never wait on the vector engine
                                nc.vector.reciprocal(recip[:, 0:512],
                                                     rs_c[0][:])
                    for c, (c0, c1) in enumerate(((0, 512), (512, 1024))):
                        if c == 1 or not causal:
                            nc.vector.reciprocal(recip[:, c0:c1], rs_c[c][:])
                        po = psum.tile([128, 512], f32, tag=f"b{4 + c}",
                                       name=f"po_{b}_{h}_{c}")
                        js = [j for j in range(NTQ)
                              if (D * j if causal else 0) < c1]
                        for j in js:
                            lo = max(D * j, c0) if causal else c0
                            nc.tensor.matmul(
                                po[:, lo - c0:c1 - c0],
                                lhsT=vT[:, j * D:(j + 1) * D],
                                rhs=pT[:, j, lo:c1],
                                start=(j == 0), stop=(j == js[-1]))
                        nc.vector.tensor_tensor(
                            oT_sb[h][b][:, c0:c1], po[:],
                            recip[:, c0:c1], mult)
                    if b == 1:
                        emit_groups(3)
            emit_groups(NG, deep=True)

    with tile.TileContext(nc) as tc:
        with tc.tile_pool(name="consts", bufs=1) as consts:
            cosT_sb = consts.tile([D, T], f32)
            sinS_sb = consts.tile([D, T], f32)
            triu_sb = consts.tile([D, D], dt_store)
            ones_sb = consts.tile([D, D], dt_store)

            def issue_consts():
                nc.sync.dma_start(cosT_sb[:], cosT_d.ap()[:])
                nc.sync.dma_start(sinS_sb[:], sinS_d.ap()[:])
                nc.sync.dma_start(triu_sb[:], triu_d.ap()[:])
                nc.sync.dma_start(ones_sb[:], ones1_d.ap()[:])

            with tc.tile_pool(name="persist", bufs=1) as persist, \
                 tc.tile_pool(name="pT", bufs=2) as ppool, \
                 tc.tile_pool(name="smx", bufs=2) as spool, \
                 tc.tile_pool(name="psum", bufs=1, space="PSUM") as psum:
                qkvT_sb = [[persist.tile([128, T], dt_store,
                                         tag=f"qkv_{ft}_{bb}",
                                         name=f"qkvT_{ft}_{bb}")
                            for bb in range(B)] for ft in range(FT)]
                v_sb = [persist.tile([128, T], dt_store,
                                     tag=f"v_{bb}", name=f"v_{bb}")
                        for bb in range(B)]
                if 1 in phases:
                    phase1(tc, psum, qkvT_sb, v_sb, issue_consts)
                if dbg:
                    with tc.tile_pool(name="dbg", bufs=2) as dpool:
                        for ft in range(FT):
                            for bb in range(B):
                                t = dpool.tile([128, T], f32, tag="d")
                                nc.vector.tensor_copy(t[:], qkvT_sb[ft][bb][:])
                                nc.sync.dma_start(
                                    qkv_dbg_d.ap()[ft * 128:(ft + 1) * 128,
                                                   bb * T:(bb + 1) * T],
                                    t[:])
                        for bb in range(B):
                            t = dpool.tile([128, T], f32, tag="d")
                            nc.vector.tensor_copy(t[:], v_sb[bb][:])
                            nc.sync.dma_start(
                                v_dbg_d.ap()[:, bb * NTQ * D:(bb + 1) * NTQ * D],
                                t[:])
                with tc.tile_pool(name="wo", bufs=1) as wopool, \
                     tc.tile_pool(name="persist2", bufs=1) as persist2:
                    # prefetch the Wo tiles while phase 2 runs (DMA idle)
                    woT_r = woT_d.ap().rearrange("(ht p) e -> p ht e", p=128)
                    w2 = []
                    for eh in range(2):
                        for ht in range(QH):
                            w2t = wopool.tile([128, 2048], dt_store,
                                              tag=f"w2_{eh}_{ht}",
                                              name=f"w2_{eh}_{ht}")
                            nc.sync.dma_start(
                                w2t[:],
                                woT_r[:, ht, eh * 2048:(eh + 1) * 2048])
                            w2.append(w2t)
                    oT_sb = [[persist2.tile([128, T], dt_store,
                                            tag=f"oT_{hh}_{bb}",
                                            name=f"oT_{hh}_{bb}")
                              for bb in range(B)] for hh in range(QH)]
                    if 2 in phases:
                        phase23(tc, psum, ppool, spool, qkvT_sb, v_sb,
                                oT_sb, w2)

    nc.compile()
    return nc


# ---------------------------------------------------------------- runner

class _Runner:
    """Compile once, keep a no-donation jitted SPMD callable."""

    def __init__(self, causal=True, phases=(1, 2, 3)):
        import jax
        from jax.sharding import Mesh, PartitionSpec
        try:
            from jax.experimental.shard_map import shard_map
        except ImportError:  # newer jax
            from jax.sharding import shard_map
        from concourse import mybir
        from concourse.bass2jax import (_bass_exec_p, install_neuronx_cc_hook,
                                        partition_id_tensor)

        self.jax = jax
        self.nc = _build_nc(causal=causal, phases=phases)
        nc = self.nc
        install_neuronx_cc_hook()

        partition_name = (nc.partition_id_tensor.name
                          if nc.partition_id_tensor else None)
        in_names, out_names, out_avals, zero_outs = [], [], [], []
        for alloc in nc.m.functions[0].allocations:
            if not isinstance(alloc, mybir.MemoryLocationSet):
                continue
            name = alloc.memorylocations[0].name
            if alloc.kind == "ExternalInput":
                if name != partition_name:
                    in_names.append(name)
            elif alloc.kind == "ExternalOutput":
                out_names.append(name)
                shape = tuple(alloc.tensor_shape)
                dtype = mybir.dt.np(alloc.dtype)
                out_avals.append(jax.core.ShapedArray(shape, dtype))
                zero_outs.append(np.zeros(shape, dtype))
        self.in_names, self.out_names = in_names, out_names
        self.zero_outs = zero_outs
        n_params = len(in_names)
        in_names_all = list(in_names) + list(out_names)
        if partition_name is not None:
            in_names_all.append(partition_name)

        def _body(*args):
            operands = list(args)
            if partition_name is not None:
                operands.append(partition_id_tensor())
            outs = _bass_exec_p.bind(
                *operands, out_avals=tuple(out_avals),
                in_names=tuple(in_names_all), out_names=tuple(out_names),
                lowering_input_output_aliases=(),
                sim_require_finite=True, sim_require_nnan=True, nc=nc)
            return tuple(outs)

        devices = jax.devices()[:M]
        assert len(devices) == M, f"need {M} cores, found {len(jax.devices())}"
        mesh = Mesh(np.asarray(devices), ("core",))
        self.mesh = mesh
        in_specs = (PartitionSpec("core"),) * (n_params + len(out_names))
        out_specs = (PartitionSpec("core"),) * len(out_names)
        # Donate the output-shaped args: the NEFF fully overwrites every
        # output tensor, so we ping-pong the previous call's outputs in as
        # the next call's donated output buffers.
        donate = tuple(range(n_params, n_params + len(out_names)))
        self.fn = jax.jit(
            shard_map(_body, mesh=mesh, in_specs=in_specs,
                      out_specs=out_specs, check_rep=False),
            keep_unused=True, donate_argnums=donate)

    def put_args(self, in_maps):
        jax = self.jax
        from jax.sharding import NamedSharding, PartitionSpec
        sh = NamedSharding(self.mesh, PartitionSpec("core"))
        concat_in = [np.concatenate([in_maps[c][nm] for c in range(M)], axis=0)
                     for nm in self.in_names]
        args = [jax.device_put(x, sh) for x in concat_in]
        self._outbufs = [
            jax.device_put(np.zeros((M * z.shape[0], *z.shape[1:]), z.dtype), sh)
            for z in self.zero_outs]
        return args

    def run(self, args):
        outs = self.fn(*args, *self._outbufs)
        self.jax.block_until_ready(outs)
        self._outbufs = list(outs)   # donated ping-pong
        return outs

    def gather(self, outs):
        """Sum the 8 partials of 'outp' -> full [B,T,H,D] output."""
        i = self.out_names.index("outp")
        arr = np.asarray(outs[i]).reshape(M, BT, CD)
        return arr.astype(np.float32).sum(0).reshape(B, T, H, D)


_RUNNERS = {}


def _get_runner(causal=True):
    if causal not in _RUNNERS:
        _RUNNERS[causal] = _Runner(causal=causal)
    return _RUNNERS[causal]


def _mask_kind(mask_w, mask_b):
    tril = np.tril(np.ones((T, T), np.float32))
    if (np.array_equal(mask_w[0, 0], tril)
            and np.allclose(mask_b[0, 0], (1.0 - tril) * NEG)):
        return "causal"
    if (mask_w == 1.0).all() and (mask_b == 0.0).all():
        return "allpass"
    return "other"


def _numpy_fallback(stm, Wq, Wk, Wv, Wo, cos, sin, mask_w, mask_b):
    x = stm.reshape(B, T, H * D).astype(np.float32)
    q = (x @ Wq.T).reshape(B, T, H, D)
    k = (x @ Wk.T).reshape(B, T, KV, D)
    v = (x @ Wv.T).reshape(B, T, KV, D)
    k = np.repeat(k, H // KV, axis=2)
    v = np.repeat(v, H // KV, axis=2)

    def rope(t):
        half = D // 2
        t2 = np.concatenate([-t[..., half:], t[..., :half]], -1)
        return t * cos + t2 * sin

    q, k = rope(q), rope(k)
    attn = np.einsum("bqhd,bkhd->bhqk", q, k).astype(np.float32) * SCALE
    attn = attn * mask_w + mask_b
    attn = attn - attn.max(-1, keepdims=True)
    attn = np.exp(attn)
    attn = attn / attn.sum(-1, keepdims=True)
    o = np.einsum("bhqk,bkhd->bqhd", attn, v).astype(np.float32)
    return (o.reshape(B, T, H * D) @ Wo.T).reshape(B, T, H, D)


def kernel(stm, Wq, Wk, Wv, Wo, cos, sin, mask_w, mask_b):
    stm = np.asarray(stm, np.float32)
    Wq, Wk, Wv, Wo = (np.asarray(a, np.float32) for a in (Wq, Wk, Wv, Wo))
    cos, sin = np.asarray(cos, np.float32), np.asarray(sin, np.float32)
    mask_w, mask_b = (np.asarray(a, np.float32) for a in (mask_w, mask_b))

    kind = _mask_kind(mask_w, mask_b)
    if kind == "other":
        return _numpy_fallback(stm, Wq, Wk, Wv, Wo, cos, sin, mask_w, mask_b)

    runner = _get_runner(causal=(kind == "causal"))
    in_maps = host_prep(stm, Wq, Wk, Wv, Wo, cos, sin, mask_b)
    args = runner.put_args(in_maps)
    outs = runner.run(args)
    return runner.gather(outs)



# revision 7
# speedup vs baseline: 1.1978x; 1.1978x over previous
"""Mistral GQA self-attention block on 8 Trainium2 NeuronCores (Bass/Tile).

Sharding: tensor-parallel over heads. Core m owns q-heads 4m..4m+3 and
kv-head m (GQA group-aligned), Wq/Wk/Wv column-sharded, Wo row-sharded.
Each core computes a full-size [B*T, H*D] partial of the output
projection; the host sums the 8 partials (the Wo row-parallel reduce).

All four projections (Q/K/V and the output projection) run as
error-compensated fp8e4 DoubleRow matmuls: each operand A is split
host-side (device-side for the attention output) into A_hi = fp8(s*A)
and A_lo = fp8(s*A - A_hi).  The product A.T@B is then

    hi.T@hi  (K=256 DoubleRow chains over k-block pairs)
  + [hi.T@lo + lo.T@hi]  (one DoubleRow instruction per k-block: the
    two cross products ride the two DoubleRow subtiles)

which costs 0.75x the fp16 matmul time at the cost model's 0.5
cycles/row DoubleRow rate, with ~fp16-class accuracy (the dropped
lo.T@lo term is O(2^-8)).  hi/lo are interleaved in the operand
layouts ([..., 2, ...] axes, x-side ordered [lo, hi], w-side
[hi, lo]) so both the paired and the cross instructions slice the
same tiles and total DMA/SBUF bytes match the old fp16 layout.

Attention itself (scores, softmax, PV) stays fp16: per-element fp8
noise on q/k/p/v transfers ~1:1 into the output and would blow the
error budget, while the projections' noise is killed by compensation.

Per-core kernel layout (feature-major, [feature, token] for q/k):
  phase 1: QKV projection. q heads + k head come out feature-major
           with RoPE applied in place (partition-swap copies plus a
           sign-folded sin table).  v is produced directly in [t, d]
           token-major layout (x sub-tile stationary).  DMAs are
           batched in multi-k-tile chunks (even-sized so DoubleRow
           k-pairs never span chunk tiles).
  phase 2+3 (merged, shared 8-bank psum tag pool):
           attention per (batch, head).  Scores are computed
           TRANSPOSED (st[tk, tq] = k_tile.T @ q); exp'd
           probabilities land in the [tk, tq] layout the PV matmul
           needs.  Causal diagonal blocks masked by a 0/1-triangle
           multiply on the Pool engine.  The softmax denominator is
           accumulated with a (1/16-scaled) ones-vector matmul and
           applied to the output tile, which is then split hi/lo to
           fp8 for the DoubleRow output projection, interleaved into
           the b=1 attention stream as PE filler.
"""

import os
import sys

import numpy as np

for _p in ("/opt/trn_rl_repo", "/root/.axon_site/_ro/trn_rl_repo"):
    if os.path.isdir(_p):
        if _p not in sys.path:
            sys.path.insert(0, _p)
        break

import ml_dtypes  # noqa: E402

B, T, H, D = 2, 1024, 32, 128
KV = 8
M = 8                 # cores
QH = H // M           # q heads per core
FT = QH + 1           # feature-major tiles per core: 4 q, 1 k (v separate)
CD = H * D            # contraction dim 4096
CT = CD // 128        # 32 c-tiles
BT = B * T            # 2048 tokens
NTQ = T // D          # 8 tq/tk tiles per batch
NEG = -1e9
SCALE = 1.0 / np.sqrt(D)

MM_MODE = os.environ.get("BASS_MM_DTYPE", "f8dr")

E4 = ml_dtypes.float8_e4m3
SXS = 16.0            # x fp8 scale
SWS = 1024.0          # weight fp8 scale (w sigma = 1/64)
SOS = 16.0            # attention-output fp8 scale (via 1/16 ones)
PROJ_DESCALE = 1.0 / (SXS * SWS)
OPROJ_DESCALE = 1.0 / (SOS * SWS)

# k-tile chunk sizes for the batched weight/x DMAs (sum = CT).  All even
# so DoubleRow k-block pairs (2c, 2c+1) stay within one chunk tile.
CHUNKS = (2, 2, 2, 2, 4, 4, 4, 4, 4, 4)
_CH_OFF = [sum(CHUNKS[:i]) for i in range(len(CHUNKS))]
N_WARM = int(os.environ.get("BASS_N_WARM", "28"))


# ---------------------------------------------------------------- host prep

def _split8(a, s):
    """Split s*a into fp8e4 hi + lo parts (lo captures the hi residual)."""
    a = np.ascontiguousarray(np.asarray(a, np.float32)) * np.float32(s)
    hi = a.astype(E4)
    lo = (a - hi.astype(np.float32)).astype(E4)
    return hi, lo


def host_prep(stm, Wq, Wk, Wv, Wo, cos, sin, mask_b):
    """Build the 8 per-core input maps."""
    x = np.asarray(stm, np.float32).reshape(BT, CD)
    xhi, xlo = _split8(x.T, SXS)                         # [4096, 2048]
    # (k i p) t layout: per k-block, a 128-row lo block then a hi block --
    # keeps the DMA access patterns 3-dimensional (k and i merge)
    xT2 = np.ascontiguousarray(
        np.stack([xlo.reshape(CT, 128, BT), xhi.reshape(CT, 128, BT)],
                 axis=1).reshape(2 * CD, BT))
    cosT = np.ascontiguousarray(cos[0, :, 0, :].T, np.float32)   # [128, 1024]
    sinS = np.ascontiguousarray(sin[0, :, 0, :].T, np.float32).copy()
    sinS[:D // 2] = -sinS[:D // 2]   # sign of the half-rotation folded in
    triu = np.triu(np.ones((D, D), np.float32)).astype(np.float16)
    ones1 = (np.ones((D, D), np.float32) / SOS).astype(np.float16)

    in_maps = []
    for m in range(M):
        wq = Wq[m * QH * D:(m + 1) * QH * D]             # [512, 4096]
        wk = Wk[m * D:(m + 1) * D]                       # [128, 4096]
        wv = Wv[m * D:(m + 1) * D]                       # [128, 4096]
        whi, wlo = _split8(np.concatenate([wq, wk, wv], 0).T, SWS)
        F = (FT + 1) * D
        wqkvT2 = np.ascontiguousarray(          # (k i p) f, i = [hi, lo]
            np.stack([whi.reshape(CT, 128, F), wlo.reshape(CT, 128, F)],
                     axis=1).reshape(2 * CD, F))
        ohi, olo = _split8(Wo[:, m * QH * D:(m + 1) * QH * D].T, SWS)
        woT2 = np.ascontiguousarray(            # (ht i p) e, i = [hi, lo]
            np.stack([ohi.reshape(QH, 128, CD), olo.reshape(QH, 128, CD)],
                     axis=1).reshape(2 * QH * D, CD))
        in_maps.append({
            "xT": xT2, "wqkvT": wqkvT2, "woT": woT2,
            "cosT": cosT, "sinS": sinS, "triu": triu,
            "ones1": ones1,
        })
    return in_maps


# ---------------------------------------------------------------- bass prog

def _build_nc(causal=True, phases=(1, 2, 3)):
    import concourse.tile as tile
    from concourse import bacc, mybir

    dt_store = mybir.dt.float16
    e4 = mybir.dt.float8e4
    f32 = mybir.dt.float32
    f16 = mybir.dt.float16
    DR = mybir.MatmulPerfMode.DoubleRow

    nc = bacc.Bacc("TRN2", target_bir_lowering=False, debug=False)

    xT_d = nc.dram_tensor("xT", [2 * CD, BT], e4, kind="ExternalInput")
    wqkvT_d = nc.dram_tensor("wqkvT", [2 * CD, (FT + 1) * D], e4,
                             kind="ExternalInput")
    woT_d = nc.dram_tensor("woT", [2 * QH * D, CD], e4, kind="ExternalInput")
    cosT_d = nc.dram_tensor("cosT", [D, T], f32, kind="ExternalInput")
    sinS_d = nc.dram_tensor("sinS", [D, T], f32, kind="ExternalInput")
    triu_d = nc.dram_tensor("triu", [D, D], dt_store, kind="ExternalInput")
    ones1_d = nc.dram_tensor("ones1", [D, D], dt_store, kind="ExternalInput")
    outp_d = nc.dram_tensor("outp", [BT, CD], f16, kind="ExternalOutput")
    dbg = bool(os.environ.get("BASS_DEBUG_QKV"))
    if dbg:
        qkv_dbg_d = nc.dram_tensor("qkv_dbg", [FT * 128, B * T], f32,
                                   kind="ExternalOutput")
        v_dbg_d = nc.dram_tensor("v_dbg", [128, B * T], f32,
                                 kind="ExternalOutput")

    add = mybir.AluOpType.add
    mult = mybir.AluOpType.mult
    sub = mybir.AluOpType.subtract
    Exp = mybir.ActivationFunctionType.Exp
    half = D // 2

    def chunks_for_j(j):
        """Valid tq chunk ranges [(lo, hi)] for k-tile j (<=512 wide,
        psum-bank aligned ends)."""
        if not causal:
            return [(0, 512), (512, 1024)]
        w0 = D * j
        out = []
        if w0 < 512:
            out.append((w0, 512))
        out.append((max(512, w0), 1024))
        return out

    def _ci(k):
        ci = 0
        while k >= _CH_OFF[ci] + CHUNKS[ci]:
            ci += 1
        return ci

    def phase1(tc, psum, qkvT_sb, v_sb, issue_consts):
        with tc.tile_pool(name="wqkv", bufs=1) as wpool, \
             tc.tile_pool(name="xin", bufs=2) as xpool, \
             tc.tile_pool(name="rope", bufs=1) as rpool:
            wqkvT_r = wqkvT_d.ap().rearrange("(k i p) f -> p k i f",
                                             i=2, p=128)
            xT_r = xT_d.ap().rearrange("(k i p) t -> p k i t", i=2, p=128)
            w_c = [wpool.tile([128, n, 2, (FT + 1) * D], e4,
                              tag=f"w{ci}", name=f"w_{ci}")
                   for ci, n in enumerate(CHUNKS)]

            # PE p-state warmup: spin on a zeroed tile into a scratch psum
            # bank while the first DMAs land, so the tensor engine is at
            # full clock when real data arrives (idle resets the ramp)
            warm = rpool.tile([128, 128], dt_store, tag="warm")
            nc.vector.memset(warm[:], 0)
            wps = psum.tile([128, 512], f32, tag="b6", name="warmup_ps")
            for _ in range(N_WARM):
                nc.tensor.matmul(wps[:, 0:128], lhsT=warm[:], rhs=warm[:],
                                 start=True, stop=True)

            def issue_x(tb):
                xc = []
                for ci, n in enumerate(CHUNKS):
                    t = xpool.tile([128, n, 2, 512], e4, tag=f"x{ci}",
                                   name=f"x_{ci}_{tb}")
                    nc.sync.dma_start(
                        t[:], xT_r[:, _CH_OFF[ci]:_CH_OFF[ci] + n, :,
                                   tb * 512:(tb + 1) * 512])
                    xc.append(t)
                return xc

            # cross slices: w [hi, lo], x [lo, hi]; pair slices: k-pair
            # (2c, 2c+1) at w-hi (i=0) / x-hi (i=1)
            def w_cross(k, ft):
                ci = _ci(k)
                return w_c[ci][:, k - _CH_OFF[ci], :, ft * D:(ft + 1) * D]

            def w_pair(c2, ft):
                k = 2 * c2
                ci = _ci(k)
                o = k - _CH_OFF[ci]
                return w_c[ci][:, o:o + 2, 0, ft * D:(ft + 1) * D]

            def x_cross(xc, k, lo=0, hi=512):
                ci = _ci(k)
                return xc[ci][:, k - _CH_OFF[ci], :, lo:hi]

            def x_pair(xc, c2, lo=0, hi=512):
                k = 2 * c2
                ci = _ci(k)
                o = k - _CH_OFF[ci]
                return xc[ci][:, o:o + 2, 1, lo:hi]

            def mm_v(pv, xc):
                # one accumulation chain at a time: concurrent psum
                # accumulation groups within one bank are illegal
                for i in range(4):
                    lo, hi = i * D, (i + 1) * D
                    for k in range(CT):
                        nc.tensor.matmul(
                            pv[:, lo:hi], lhsT=x_cross(xc, k, lo, hi),
                            rhs=w_cross(k, FT),
                            start=(k == 0), stop=False, perf_mode=DR)
                        if k % 2 == 1:
                            nc.tensor.matmul(
                                pv[:, lo:hi], lhsT=x_pair(xc, k // 2, lo, hi),
                                rhs=w_pair(k // 2, FT),
                                start=False, stop=(k == CT - 1), perf_mode=DR)

            def mm_ft_k(ps, xc, k, ft):
                nc.tensor.matmul(
                    ps[ft][:], lhsT=w_cross(k, ft), rhs=x_cross(xc, k),
                    start=(k == 0), stop=False, perf_mode=DR)
                if k % 2 == 1:
                    nc.tensor.matmul(
                        ps[ft][:], lhsT=w_pair(k // 2, ft),
                        rhs=x_pair(xc, k // 2),
                        start=False, stop=(k == CT - 1), perf_mode=DR)

            def rope_qk(tb, ps):
                """Drain the q/k psum accumulators (with the fp8 descale)
                and apply RoPE.  Emitted BEFORE the v chains so Act/DVE
                drain the banks while the PE runs the v matmuls."""
                t0 = (tb % 2) * 512
                csl = cosT_sb[:, t0:t0 + 512]
                ssl = sinS_sb[:, t0:t0 + 512]
                bb, tsl = tb // 2, slice(t0, t0 + 512)
                qraw = [rpool.tile([128, 512], f32, tag=f"qraw{ft}",
                                   name=f"qraw_{ft}_{tb}")
                        for ft in range(FT)]
                for ft in range(FT):
                    nc.scalar.mul(qraw[ft][:], ps[ft][:], PROJ_DESCALE)
                for ft in range(FT):
                    dtile = qkvT_sb[ft][bb]
                    dst = dtile[:, tsl]
                    q = qraw[ft]
                    # half-rotation via partition-swap copies, then
                    # dst = swap(q) * sinS + q * cos -- all on the vector
                    # engine so the chain has no cross-engine ping-pong
                    nc.vector.tensor_copy(dtile[0:half, tsl], q[half:128, :])
                    nc.vector.tensor_copy(dtile[half:128, tsl], q[0:half, :])
                    tmp = rpool.tile([128, 512], f32, tag="rtmp")
                    nc.vector.tensor_tensor(dst, dst, ssl, mult)
                    nc.vector.tensor_tensor(tmp[:], q[:], csl, mult)
                    nc.vector.tensor_tensor(dst, dst, tmp[:], add)

            def drain_v(tb, pv):
                tsl = slice((tb % 2) * 512, (tb % 2) * 512 + 512)
                nc.scalar.mul(v_sb[tb // 2][:, tsl], pv[:], PROJ_DESCALE)

            for tb in range(BT // 512):
                ps = [psum.tile([128, 512], f32, tag=f"b{ft}",
                                name=f"ps_qkv{ft}_{tb}")
                      for ft in range(FT)]
                pv = psum.tile([128, 512], f32, tag="b5", name=f"ps_v_{tb}")
                if tb == 0:
                    # interleave w / x chunk DMAs, then consts + wo
                    for ci, n in enumerate(CHUNKS):
                        nc.sync.dma_start(
                            w_c[ci][:], wqkvT_r[:, _CH_OFF[ci]:_CH_OFF[ci] + n])
                        xt = xpool.tile([128, n, 2, 512], e4, tag=f"x{ci}",
                                        name=f"x_{ci}_0")
                        nc.sync.dma_start(
                            xt[:], xT_r[:, _CH_OFF[ci]:_CH_OFF[ci] + n, :,
                                        0:512])
                        if ci == 0:
                            xc = []
                        xc.append(xt)
                    issue_consts()
                    # k-outer while the chunk DMAs stream in
                    for k in range(CT):
                        for ft in range(FT):
                            mm_ft_k(ps, xc, k, ft)
                else:
                    # ft-outer: each chain runs ~5us, hiding the psum
                    # drain of the previous tb's rope
                    for ft in range(FT):
                        for k in range(CT):
                            mm_ft_k(ps, xc, k, ft)
                rope_qk(tb, ps)
                mm_v(pv, xc)
                if tb < BT // 512 - 1:
                    xc = issue_x(tb + 1)
                drain_v(tb, pv)

    def phase23(tc, psum, ppool, spool, qkvT_sb, v_sb, oT2_sb, w2):
        with tc.tile_pool(name="oout", bufs=4) as opool:
            # --- output-projection groups: [128,512] psum DoubleRow chains
            # (2 hi-pair + 4 cross instructions over the 4 head-tiles),
            # emitted as PE filler inside phase 2 (b=1) and in bulk after.
            NG = (BT // 128) * 8           # (tt) x (eh, g2) groups
            gstate = {"g": 0, "ot": None, "c": 0}
            tags4 = ["b6", "b7", "b4", "b5"]
            tags8 = [f"b{i}" for i in range(8)]

            def emit_groups(count, deep=False):
                tags = tags8 if deep else tags4
                for _ in range(count):
                    gi = gstate["g"]
                    if gi >= NG:
                        return
                    gstate["g"] = gi + 1
                    tt, rem = divmod(gi, 8)
                    eh, g2 = divmod(rem, 4)
                    if g2 == 0:
                        gstate["ot"] = opool.tile([128, 2048], f16, tag="ot",
                                                  name=f"ot_{tt}_{eh}")
                    ot = gstate["ot"]
                    tag = tags[gstate["c"] % len(tags)]
                    gstate["c"] += 1
                    pps = psum.tile([128, 512], f32, tag=tag,
                                    name=f"ps3_{tt}_{eh}_{g2}")
                    bb, tq = tt // NTQ, tt % NTQ
                    tqsl = slice(tq * D, (tq + 1) * D)
                    osl = slice(g2 * 512, (g2 + 1) * 512)
                    oT2 = oT2_sb[bb]
                    for hp in range(2):
                        nc.tensor.matmul(
                            pps[:],
                            lhsT=oT2[:, 2 * hp:2 * hp + 2, 1, tqsl],
                            rhs=w2[eh][:, 2 * hp:2 * hp + 2, 0, osl],
                            start=(hp == 0), stop=False, perf_mode=DR)
                    for ht in range(QH):
                        nc.tensor.matmul(
                            pps[:], lhsT=oT2[:, ht, :, tqsl],
                            rhs=w2[eh][:, ht, :, osl],
                            start=False, stop=(ht == QH - 1), perf_mode=DR)
                    if gi % 2 == 0:
                        nc.vector.tensor_scalar_mul(ot[:, osl], pps[:],
                                                    OPROJ_DESCALE)
                    else:
                        nc.scalar.mul(ot[:, osl], pps[:], OPROJ_DESCALE)
                    if gi >= NG - 4:
                        # tail groups: per-group DMA so the final transfer
                        # (which gates the drain) is 4x smaller
                        nc.sync.dma_start(
                            outp_d.ap()[tt * 128:(tt + 1) * 128,
                                        eh * 2048 + g2 * 512:
                                        eh * 2048 + (g2 + 1) * 512],
                            ot[:, osl])
                    elif g2 == 3:
                        nc.sync.dma_start(
                            outp_d.ap()[tt * 128:(tt + 1) * 128,
                                        eh * 2048:(eh + 1) * 2048],
                            ot[:])

            for b in range(B):
                vT = v_sb[b]
                for h in range(QH):
                    qsl = qkvT_sb[h][b][:]
                    ksl = qkvT_sb[QH][b][:]
                    rs_c = [psum.tile([128, 512], f32, tag=f"b{6 + c}",
                                      name=f"rs_{b}_{h}_{c}")
                            for c in range(2)]
                    pT = ppool.tile([128, NTQ, T], dt_store, tag="pT")
                    st_tags = ("b0", "b1", "b2", "b3")
                    nst = 0
                    recip = spool.tile([128, T], f32, tag="recip")
                    for j in range(NTQ):
                        for ci, (lo, hi) in enumerate(chunks_for_j(j)):
                            W = hi - lo
                            st = psum.tile([128, 512], f32,
                                           tag=st_tags[nst % 4],
                                           name=f"st_{b}_{h}_{j}_{ci}")
                            nst += 1
                            diag = causal and ci == 0
                            nc.tensor.matmul(
                                st[:, :W],
                                lhsT=ksl[:, j * D:(j + 1) * D],
                                rhs=qsl[:, lo:hi],
                                start=True, stop=True)
                            nc.scalar.activation(
                                pT[:, j, lo:hi], st[:, :W], Exp,
                                scale=float(SCALE))
                            if diag:
                                # zero the causally-invalid lower triangle
                                # of the diagonal block on the (idle) Pool
                                # engine instead of a PE -1e9 preload
                                nc.gpsimd.tensor_tensor(
                                    pT[:, j, lo:lo + D], pT[:, j, lo:lo + D],
                                    triu_sb[:], mult)
                            last_j = (min(NTQ, hi // D) - 1) if causal \
                                else NTQ - 1
                            c = 0 if hi <= 512 else 1
                            nc.tensor.matmul(
                                rs_c[c][:, lo - c * 512:hi - c * 512],
                                lhsT=ones_sb[:],
                                rhs=pT[:, j, lo:hi],
                                start=(j == 0),
                                stop=(j == last_j))
                            if causal and c == 0 and j == last_j:
                                # c0's denominator is final here; compute its
                                # reciprocal now so the po matmuls after the
                                # j loop never wait on the vector engine
                                nc.vector.reciprocal(recip[:, 0:512],
                                                     rs_c[0][:])
                    for c, (c0, c1) in enumerate(((0, 512), (512, 1024))):
                        if c == 1 or not causal:
                            nc.vector.reciprocal(recip[:, c0:c1], rs_c[c][:])
                        po = psum.tile([128, 512], f32, tag=f"b{4 + c}",
                                       name=f"po_{b}_{h}_{c}")
                        js = [j for j in range(NTQ)
                              if (D * j if causal else 0) < c1]
                        for j in js:
                            lo = max(D * j, c0) if causal else c0
                            nc.tensor.matmul(
                                po[:, lo - c0:c1 - c0],
                                lhsT=vT[:, j * D:(j + 1) * D],
                                rhs=pT[:, j, lo:c1],
                                start=(j == 0), stop=(j == js[-1]))
                        # t16 = po * (16/denom); split to fp8 hi/lo for the
                        # DoubleRow output projection
                        t16 = spool.tile([128, 512], f16, tag=f"t16_{c}")
                        nc.vector.tensor_tensor(
                            t16[:], po[:], recip[:, c0:c1], mult)
                        nc.scalar.copy(oT2_sb[b][:, h, 1, c0:c1], t16[:])
                        nc.vector.tensor_tensor(
                            oT2_sb[b][:, h, 0, c0:c1], t16[:],
                            oT2_sb[b][:, h, 1, c0:c1], sub)
                    if b == 1:
                        emit_groups(3)
            emit_groups(NG, deep=True)

    with tile.TileContext(nc) as tc:
        with tc.tile_pool(name="consts", bufs=1) as consts:
            cosT_sb = consts.tile([D, T], f32)
            sinS_sb = consts.tile([D, T], f32)
            triu_sb = consts.tile([D, D], dt_store)
            ones_sb = consts.tile([D, D], dt_store)

            def issue_consts():
                nc.sync.dma_start(cosT_sb[:], cosT_d.ap()[:])
                nc.sync.dma_start(sinS_sb[:], sinS_d.ap()[:])
                nc.sync.dma_start(triu_sb[:], triu_d.ap()[:])
                nc.sync.dma_start(ones_sb[:], ones1_d.ap()[:])

            with tc.tile_pool(name="persist", bufs=1) as persist, \
                 tc.tile_pool(name="pT", bufs=2) as ppool, \
                 tc.tile_pool(name="smx", bufs=2) as spool, \
                 tc.tile_pool(name="psum", bufs=1, space="PSUM") as psum:
                qkvT_sb = [[persist.tile([128, T], dt_store,
                                         tag=f"qkv_{ft}_{bb}",
                                         name=f"qkvT_{ft}_{bb}")
                            for bb in range(B)] for ft in range(FT)]
                v_sb = [persist.tile([128, T], dt_store,
                                     tag=f"v_{bb}", name=f"v_{bb}")
                        for bb in range(B)]
                if 1 in phases:
                    phase1(tc, psum, qkvT_sb, v_sb, issue_consts)
                if dbg:
                    with tc.tile_pool(name="dbg", bufs=2) as dpool:
                        for ft in range(FT):
                            for bb in range(B):
                                t = dpool.tile([128, T], f32, tag="d")
                                nc.vector.tensor_copy(t[:], qkvT_sb[ft][bb][:])
                                nc.sync.dma_start(
                                    qkv_dbg_d.ap()[ft * 128:(ft + 1) * 128,
                                                   bb * T:(bb + 1) * T],
                                    t[:])
                        for bb in range(B):
                            t = dpool.tile([128, T], f32, tag="d")
                            nc.vector.tensor_copy(t[:], v_sb[bb][:])
                            nc.sync.dma_start(
                                v_dbg_d.ap()[:, bb * NTQ * D:(bb + 1) * NTQ * D],
                                t[:])
                with tc.tile_pool(name="wo", bufs=1) as wopool, \
                     tc.tile_pool(name="persist2", bufs=1) as persist2:
                    # prefetch the Wo tiles while phase 2 runs (DMA idle)
                    woT_r = woT_d.ap().rearrange("(ht i p) e -> p ht i e",
                                                 i=2, p=128)
                    w2 = []
                    for eh in range(2):
                        w2t = wopool.tile([128, QH, 2, 2048], e4,
                                          tag=f"w2_{eh}", name=f"w2_{eh}")
                        nc.sync.dma_start(
                            w2t[:],
                            woT_r[:, :, :, eh * 2048:(eh + 1) * 2048])
                        w2.append(w2t)
                    # [:, ht, 0, :] = oT_lo, [:, ht, 1, :] = oT_hi
                    oT2_sb = [persist2.tile([128, QH, 2, T], e4,
                                            tag=f"oT2_{bb}",
                                            name=f"oT2_{bb}")
                              for bb in range(B)]
                    if 2 in phases:
                        phase23(tc, psum, ppool, spool, qkvT_sb, v_sb,
                                oT2_sb, w2)

    nc.compile()
    return nc


# ---------------------------------------------------------------- runner

class _Runner:
    """Compile once, keep a no-donation jitted SPMD callable."""

    def __init__(self, causal=True, phases=(1, 2, 3)):
        import jax
        from jax.sharding import Mesh, PartitionSpec
        try:
            from jax.experimental.shard_map import shard_map
        except ImportError:  # newer jax
            from jax.sharding import shard_map
        from concourse import mybir
        from concourse.bass2jax import (_bass_exec_p, install_neuronx_cc_hook,
                                        partition_id_tensor)

        self.jax = jax
        self.nc = _build_nc(causal=causal, phases=phases)
        nc = self.nc
        install_neuronx_cc_hook()

        partition_name = (nc.partition_id_tensor.name
                          if nc.partition_id_tensor else None)
        in_names, out_names, out_avals, zero_outs = [], [], [], []
        for alloc in nc.m.functions[0].allocations:
            if not isinstance(alloc, mybir.MemoryLocationSet):
                continue
            name = alloc.memorylocations[0].name
            if alloc.kind == "ExternalInput":
                if name != partition_name:
                    in_names.append(name)
            elif alloc.kind == "ExternalOutput":
                out_names.append(name)
                shape = tuple(alloc.tensor_shape)
                dtype = mybir.dt.np(alloc.dtype)
                out_avals.append(jax.core.ShapedArray(shape, dtype))
                zero_outs.append(np.zeros(shape, dtype))
        self.in_names, self.out_names = in_names, out_names
        self.zero_outs = zero_outs
        n_params = len(in_names)
        in_names_all = list(in_names) + list(out_names)
        if partition_name is not None:
            in_names_all.append(partition_name)

        def _body(*args):
            operands = list(args)
            if partition_name is not None:
                operands.append(partition_id_tensor())
            outs = _bass_exec_p.bind(
                *operands, out_avals=tuple(out_avals),
                in_names=tuple(in_names_all), out_names=tuple(out_names),
                lowering_input_output_aliases=(),
                sim_require_finite=True, sim_require_nnan=True, nc=nc)
            return tuple(outs)

        devices = jax.devices()[:M]
        assert len(devices) == M, f"need {M} cores, found {len(jax.devices())}"
        mesh = Mesh(np.asarray(devices), ("core",))
        self.mesh = mesh
        in_specs = (PartitionSpec("core"),) * (n_params + len(out_names))
        out_specs = (PartitionSpec("core"),) * len(out_names)
        # Donate the output-shaped args: the NEFF fully overwrites every
        # output tensor, so we ping-pong the previous call's outputs in as
        # the next call's donated output buffers.
        donate = tuple(range(n_params, n_params + len(out_names)))
        self.fn = jax.jit(
            shard_map(_body, mesh=mesh, in_specs=in_specs,
                      out_specs=out_specs, check_rep=False),
            keep_unused=True, donate_argnums=donate)

    def put_args(self, in_maps):
        jax = self.jax
        from jax.sharding import NamedSharding, PartitionSpec
        sh = NamedSharding(self.mesh, PartitionSpec("core"))
        concat_in = [np.concatenate([in_maps[c][nm] for c in range(M)], axis=0)
                     for nm in self.in_names]
        args = [jax.device_put(x, sh) for x in concat_in]
        self._outbufs = [
            jax.device_put(np.zeros((M * z.shape[0], *z.shape[1:]), z.dtype), sh)
            for z in self.zero_outs]
        return args

    def run(self, args):
        outs = self.fn(*args, *self._outbufs)
        self.jax.block_until_ready(outs)
        self._outbufs = list(outs)   # donated ping-pong
        return outs

    def gather(self, outs):
        """Sum the 8 partials of 'outp' -> full [B,T,H,D] output."""
        i = self.out_names.index("outp")
        arr = np.asarray(outs[i]).reshape(M, BT, CD)
        return arr.astype(np.float32).sum(0).reshape(B, T, H, D)


_RUNNERS = {}


def _get_runner(causal=True):
    if causal not in _RUNNERS:
        _RUNNERS[causal] = _Runner(causal=causal)
    return _RUNNERS[causal]


def _mask_kind(mask_w, mask_b):
    tril = np.tril(np.ones((T, T), np.float32))
    if (np.array_equal(mask_w[0, 0], tril)
            and np.allclose(mask_b[0, 0], (1.0 - tril) * NEG)):
        return "causal"
    if (mask_w == 1.0).all() and (mask_b == 0.0).all():
        return "allpass"
    return "other"


def _numpy_fallback(stm, Wq, Wk, Wv, Wo, cos, sin, mask_w, mask_b):
    x = stm.reshape(B, T, H * D).astype(np.float32)
    q = (x @ Wq.T).reshape(B, T, H, D)
    k = (x @ Wk.T).reshape(B, T, KV, D)
    v = (x @ Wv.T).reshape(B, T, KV, D)
    k = np.repeat(k, H // KV, axis=2)
    v = np.repeat(v, H // KV, axis=2)

    def rope(t):
        half = D // 2
        t2 = np.concatenate([-t[..., half:], t[..., :half]], -1)
        return t * cos + t2 * sin

    q, k = rope(q), rope(k)
    attn = np.einsum("bqhd,bkhd->bhqk", q, k).astype(np.float32) * SCALE
    attn = attn * mask_w + mask_b
    attn = attn - attn.max(-1, keepdims=True)
    attn = np.exp(attn)
    attn = attn / attn.sum(-1, keepdims=True)
    o = np.einsum("bhqk,bkhd->bqhd", attn, v).astype(np.float32)
    return (o.reshape(B, T, H * D) @ Wo.T).reshape(B, T, H, D)


def kernel(stm, Wq, Wk, Wv, Wo, cos, sin, mask_w, mask_b):
    stm = np.asarray(stm, np.float32)
    Wq, Wk, Wv, Wo = (np.asarray(a, np.float32) for a in (Wq, Wk, Wv, Wo))
    cos, sin = np.asarray(cos, np.float32), np.asarray(sin, np.float32)
    mask_w, mask_b = (np.asarray(a, np.float32) for a in (mask_w, mask_b))

    kind = _mask_kind(mask_w, mask_b)
    if kind == "other":
        return _numpy_fallback(stm, Wq, Wk, Wv, Wo, cos, sin, mask_w, mask_b)

    runner = _get_runner(causal=(kind == "causal"))
    in_maps = host_prep(stm, Wq, Wk, Wv, Wo, cos, sin, mask_b)
    args = runner.put_args(in_maps)
    outs = runner.run(args)
    return runner.gather(outs)
